# revision 1
# baseline (speedup 1.0000x reference)
"""AdaptiveRepVGGDW on 8 TRN2 NeuronCores — data-parallel over batch.

Per core (8 samples): channels on SBUF partitions, 2 groups of 128.
Depthwise convs: PSUM-accumulated TensorEngine matmuls with diagonal
stationary matrices diag(k_tap) against shifted-window views of zero-padded
bf16 x planes (group-1 conv3 runs on the VectorEngine concurrently).
The final BatchNorm's statistics are computed ANALYTICALLY from per-sample
raw sums (S3, S33, S5, S55, Sx, Sxx, S35, S3x, S5x) and softmax-weight
aggregates, so each group needs exactly ONE tiny AllReduce and out_pre is
never materialized — the last pass applies the fully-folded affine
directly to c3/c5/x and streams f32 to HBM. The kernel-predictor input is
AllGathered early so its whole chain hides under the convs.
"""

import numpy as np

import concourse.bass as bass
import concourse.bacc as bacc
import concourse.mybir as mybir
import concourse.tile as tile
from concourse.bass_utils import run_bass_kernel_spmd

F32 = mybir.dt.float32
BF16 = mybir.dt.bfloat16
AX = mybir.AxisListType
ALU = mybir.AluOpType
ACT = mybir.ActivationFunctionType

N_CORES = 8
B, C, H, W = 64, 256, 32, 32
BL = B // N_CORES          # 8 samples per core
HW = H * W                 # 1024
PH = PW = 36               # padded plane (pad=2 each side)
NG = 2                     # channel groups of 128
G = 128
NTOT = B * HW              # 65536 (BN sample count)
EPS = 1e-5
INV_N = 1.0 / NTOT
INV_B = 1.0 / B

# payload column layout (per group, [G, NPAY])
COLS = ["S3", "S33", "S5", "S55",          # pairs: m at 0,2 / q at 1,3
        "uS3", "vS5", "Sx",                # X3: so_ row + t3 b1 row
        "u2S3", "uvS5", "uSx",             # X1: t3 d3 row
        "uvS3", "v2S5", "vSx",             # X2: t3 d5 row
        "u2S33", "v2S55", "Sxx",           # Y1: quadratic row
        "uvS35", "uS3x", "vS5x",           # Y2: cross row
        "U1", "U2", "UV", "V1", "V2"]
NPAY = len(COLS)
CI = {n: i for i, n in enumerate(COLS)}

_BUILT = {}


def _build():
    nc = bacc.Bacc("TRN2", target_bir_lowering=False, debug=False,
                   num_devices=N_CORES)

    def inp(name, shape):
        return nc.dram_tensor(name, shape, F32, kind="ExternalInput").ap()

    x_ext = inp("x", [BL, C, H, W])
    # wp[g]: per-group packed weights [G, 46] = k3(9) | k5(25) | kp1s(4) | pv(8)
    wp_ext = [inp(f"wp{g}", [G, 46]) for g in range(NG)]
    # wq: packed misc [G, 144] = id128(128) | sel(8) | kp2t(2) | kbn(4) | id2(2)
    wq_ext = inp("wq", [G, 144])
    out_ext = nc.dram_tensor("out", [BL, C, H, W], F32,
                             kind="ExternalOutput").ap()

    RG = [list(range(N_CORES))]

    with tile.TileContext(nc) as tc:
        with (tc.tile_pool(name="big", bufs=1) as big,
              tc.tile_pool(name="small", bufs=1) as small,
              tc.tile_pool(name="tb", bufs=6) as tb,
              tc.tile_pool(name="tv", bufs=4) as tv,
              tc.tile_pool(name="psum", bufs=1, space="PSUM") as psum,
              tc.tile_pool(name="cpsum", bufs=3, space="PSUM") as cpsum,
              tc.tile_pool(name="dram", bufs=1, space="DRAM") as dram):

            # ---------------- persistent SBUF tensors ----------------
            xps = [[big.tile([G, PH, PW], BF16, tag=f"xp{g}_{b}",
                             name=f"xp{g}_{b}") for b in range(BL)]
                   for g in range(NG)]
            c3s = [[big.tile([G, HW], BF16, tag=f"c3_{g}_{b}",
                             name=f"c3_{g}_{b}") for b in range(BL)]
                   for g in range(NG)]
            c5s = [[big.tile([G, HW], BF16, tag=f"c5_{g}_{b}",
                             name=f"c5_{g}_{b}") for b in range(BL)]
                   for g in range(NG)]
            wp = [small.tile([G, 46], F32, tag=f"wp{g}", name=f"wp{g}")
                  for g in range(NG)]
            wq = small.tile([G, 144], F32, tag="wq", name="wq")
            k3sb = [wp[g][:, 0:9] for g in range(NG)]
            k5sb = [wp[g][:, 9:34] for g in range(NG)]
            kp1s = [wp[g][:, 34:38] for g in range(NG)]
            pv = [wp[g][:, 38:46] for g in range(NG)]
            id128 = wq[:, 0:128]
            selsb = wq[0:B, 128:136]
            kp2t = wq[0:4, 136:138]
            kbn = wq[0:4, 138:142]
            id2 = wq[0:2, 142:144]
            epst = small.tile([G, 1], F32, tag="epst", name="epst")
            pooled = [small.tile([G, BL], F32, tag=f"pool{g}", name=f"pool{g}")
                      for g in range(NG)]
            junk = small.tile([G, HW], BF16, tag="junk", name="junk")
            junkd = small.tile([G, HW], BF16, tag="junkd", name="junkd")
            junkf = small.tile([G, HW], F32, tag="junkf", name="junkf")

            nc.vector.memset(epst[:], EPS)

            # ---------------- load weights / params (3 packed DMAs) ------
            for g in range(NG):
                nc.sync.dma_start(out=wp[g][:], in_=wp_ext[g])
            nc.sync.dma_start(out=wq[:], in_=wq_ext)

            # diag(k_tap) stationary matrices (PE convs: g0 c3/c5, g1 c5)
            diag3 = [[small.tile([G, G], BF16, tag=f"dg3_{g}_{t}",
                                 name=f"dg3_{g}_{t}") for t in range(9)]
                     for g in range(NG)]
            diag5 = [[small.tile([G, G], BF16, tag=f"dg5_{g}_{t}",
                                 name=f"dg5_{g}_{t}") for t in range(25)]
                     for g in range(NG)]
            def build_diags(g, both):
                if both:
                    for t in range(9):
                        nc.vector.tensor_scalar(diag3[g][t][:], id128,
                                                k3sb[g][:, t:t + 1], None,
                                                ALU.mult)
                for t in range(25):
                    nc.vector.tensor_scalar(diag5[g][t][:], id128,
                                            k5sb[g][:, t:t + 1], None, ALU.mult)
            build_diags(1, True)

            # ---------------- stage padded bf16 x ----------------
            xr = x_ext.rearrange("b c h w -> c b h w")
            for g in (1, 0):
                cb = g * G
                for b in range(BL):
                    nc.gpsimd.memset(xps[g][b][:], 0.0)
                    stg = tb.tile([G, HW], F32, tag="stg", name="stg")
                    nc.sync.dma_start(out=stg[:], in_=xr[cb:cb + G, b])
                    nc.vector.tensor_copy(
                        xps[g][b][:, 2:34, 2:34],
                        stg[:].rearrange("p (h w) -> p h w", h=H))
                if g == 1:
                    build_diags(0, True)

            # ---------------- pooled + early z1 AllGather ----------------
            for g in (1, 0):
                for b in range(BL):
                    nc.scalar.activation(
                        junkf[:].rearrange("p (h w) -> p h w", h=H),
                        xps[g][b][:, 2:34, 2:34],
                        ACT.Copy, accum_out=pooled[g][:, b:b + 1])
            z1p = psum.tile([BL, 4], F32, tag="pps", name="z1p")
            for g in range(NG):
                nc.tensor.matmul(z1p[:], pooled[g][:], kp1s[g],
                                 start=(g == 0), stop=(g == NG - 1))
            z1sb = small.tile([BL, 4], F32, tag="z1sb", name="z1sb")
            nc.scalar.copy(z1sb[:], z1p[:])
            payz = dram.tile([BL * 4], F32, tag="payz", name="payz")
            gz = dram.tile([N_CORES, BL * 4], F32, tag="gz", name="gz")
            nc.sync.dma_start(out=payz[:].rearrange("(p j) -> p j", j=4),
                              in_=z1sb[:])
            nc.gpsimd.collective_compute(
                "AllGather", ALU.bypass, replica_groups=RG,
                ins=[payz[:].opt()], outs=[gz[:].opt()])

            # ---------------- kernel predictor (hidden under convs) -------
            gz_ap = gz[:].flatten()
            z1T = small.tile([4, B], F32, tag="z1T", name="z1T")
            for r in range(N_CORES):
                nc.sync.dma_start(
                    out=z1T[:, r * BL:(r + 1) * BL],
                    in_=bass.AP(tensor=gz_ap.tensor,
                                offset=gz_ap.offset + r * BL * 4,
                                ap=[[1, 4], [4, BL]]))

            def bn1d(src, n_feat, g_col, b_col):
                m = small.tile([n_feat, 1], F32, tag="p_m", name="p_m")
                nc.vector.reduce_sum(m[:], src, axis=AX.X)
                nc.vector.tensor_scalar(m[:], m[:], INV_B, None, ALU.mult)
                xc = small.tile([n_feat, B], F32, tag="p_xc", name="p_xc")
                nc.vector.tensor_scalar(xc[:], src, m[:], None, ALU.subtract)
                ssq = small.tile([n_feat, 1], F32, tag="p_ssq", name="p_ssq")
                jk = small.tile([n_feat, B], F32, tag="p_junk", name="p_junk")
                nc.scalar.activation(jk[:], xc[:], ACT.Square, accum_out=ssq[:])
                var = small.tile([n_feat, 1], F32, tag="p_var", name="p_var")
                nc.vector.tensor_scalar(var[:], ssq[:], INV_B, None, ALU.mult)
                sd = small.tile([n_feat, 1], F32, tag="p_sd", name="p_sd")
                nc.scalar.activation(sd[:], var[:], ACT.Sqrt,
                                     bias=epst[0:n_feat, :])
                rstd = small.tile([n_feat, 1], F32, tag="p_rstd", name="p_rstd")
                nc.vector.reciprocal(rstd[:], sd[:])
                seff = small.tile([n_feat, 1], F32, tag="p_seff", name="p_seff")
                nc.vector.tensor_tensor(seff[:], rstd[:],
                                        kbn[0:n_feat, g_col:g_col + 1], ALU.mult)
                return xc, seff

            xc1, seff1 = bn1d(z1T[:], 4, 0, 1)
            h = small.tile([4, B], F32, tag="p_h", name="p_h")
            nc.scalar.activation(h[:], xc1[:], ACT.Gelu, bias=kbn[0:4, 1:2],
                                 scale=seff1[:])
            lg = psum.tile([2, B], F32, tag="pps", name="lg")
            nc.tensor.matmul(lg[:], kp2t, h[:], start=True, stop=True)
            xc2, seff2 = bn1d(lg[:], 2, 2, 3)
            ln = small.tile([2, B], F32, tag="p_ln", name="p_ln")
            nc.vector.tensor_scalar(ln[:], xc2[:], seff2[:], kbn[0:2, 3:4],
                                    ALU.mult, ALU.add)
            lnT = psum.tile([B, 2], F32, tag="pps", name="lnT")
            nc.tensor.matmul(lnT[:], ln[:], id2, is_transpose=True,
                             start=True, stop=True)
            lnTs = small.tile([B, 2], F32, tag="lnTs", name="lnTs")
            nc.scalar.copy(lnTs[:], lnT[:])
            diff = small.tile([B, 1], F32, tag="p_diff", name="p_diff")
            nc.vector.tensor_tensor(diff[:], lnTs[:, 0:1], lnTs[:, 1:2],
                                    ALU.subtract)
            krs = small.tile([B, 2], F32, tag="krs", name="krs")
            nc.scalar.activation(krs[:, 0:1], diff[:], ACT.Sigmoid)
            nc.vector.tensor_scalar(krs[:, 1:2], krs[:, 0:1], -1.0, 1.0,
                                    ALU.mult, ALU.add)
            kwp = psum.tile([BL, 2], F32, tag="pps", name="kwp")
            nc.tensor.matmul(kwp[:], selsb, krs[:], start=True, stop=True)
            kwsb = small.tile([BL, 2], F32, tag="kwsb", name="kwsb")
            nc.scalar.copy(kwsb[:], kwp[:])
            kwd = dram.tile([BL, 2], F32, tag="kwd", name="kwd")
            nc.sync.dma_start(out=kwd[:], in_=kwsb[:])
            kwbc = small.tile([G, 2, BL], F32, tag="kwbc", name="kwbc")
            kwd_ap = kwd[:].flatten()
            for j in range(2):
                nc.sync.dma_start(
                    out=kwbc[:, j, :],
                    in_=bass.AP(tensor=kwd_ap.tensor, offset=kwd_ap.offset + j,
                                ap=[[0, G], [2, BL]]))
            kwu = kwbc[:, 0, :]
            kwv = kwbc[:, 1, :]
            u2 = small.tile([G, BL], F32, tag="u2", name="u2")
            v2 = small.tile([G, BL], F32, tag="v2", name="v2")
            uv = small.tile([G, BL], F32, tag="uv", name="uv")
            nc.vector.tensor_tensor(u2[:], kwu, kwu, ALU.mult)
            nc.vector.tensor_tensor(v2[:], kwv, kwv, ALU.mult)
            nc.vector.tensor_tensor(uv[:], kwu, kwv, ALU.mult)

            # ---------------- convs + per-sample raw sums -----------------
            def conv_pe(g, which, ssc, sqc, bps=None):
                taps = 9 if which == "c3" else 25
                kk = 3 if which == "c3" else 5
                off = 1 if which == "c3" else 0
                dgs = diag3[g] if which == "c3" else diag5[g]
                dst = c3s[g] if which == "c3" else c5s[g]
                for bp in (bps if bps is not None else range(0, BL, 2)):
                    for b in (bp, bp + 1):
                        ps = cpsum.tile([G, HW], F32, tag="cps", name="cps")
                        for t in range(taps):
                            dh, dw = divmod(t, kk)
                            for half in range(2):
                                r0 = half * 16
                                rhs = xps[g][b][:,
                                               off + dh + r0:off + dh + r0 + 16,
                                               off + dw:off + dw + W]
                                nc.tensor.matmul(
                                    ps[:, half * 512:(half + 1) * 512],
                                    dgs[t][:], rhs,
                                    start=(t == 0), stop=(t == taps - 1),
                                    skip_group_check=True)
                        nc.scalar.activation(dst[b][:], ps[:], ACT.Copy,
                                             accum_out=ssc[:, b:b + 1])
                        nc.scalar.activation(junk[:], dst[b][:], ACT.Square,
                                             accum_out=sqc[:, b:b + 1])

            def conv_dve(g, which, ssc, sqc, bs=None):
                taps = 9 if which == "c3" else 25
                kk = 3 if which == "c3" else 5
                off = 1 if which == "c3" else 0
                ksb = k3sb[g] if which == "c3" else k5sb[g]
                dst = c3s[g] if which == "c3" else c5s[g]
                for b in (bs if bs is not None else range(BL)):
                    d3 = dst[b][:].rearrange("p (h w) -> p h w", h=H)
                    for t in range(taps):
                        dh, dw = divmod(t, kk)
                        xv = xps[g][b][:, off + dh:off + dh + H,
                                       off + dw:off + dw + W]
                        kap = ksb[:, t:t + 1]
                        if t == 0:
                            nc.vector.tensor_scalar(d3, xv, kap, None, ALU.mult)
                        elif t < taps - 1:
                            nc.vector.scalar_tensor_tensor(
                                d3, xv, kap, d3, ALU.mult, ALU.add)
                        else:
                            nc.vector.scalar_tensor_tensor(
                                d3, xv, kap, d3, ALU.mult, ALU.add,
                                accum_out=ssc[:, b:b + 1])
                    nc.scalar.activation(junk[:], dst[b][:], ACT.Square,
                                         accum_out=sqc[:, b:b + 1])

            # per-sample raw sums per group
            sS3 = [small.tile([G, BL], F32, tag=f"sS3_{g}", name=f"sS3_{g}") for g in range(NG)]
            sS33 = [small.tile([G, BL], F32, tag=f"sS33_{g}", name=f"sS33_{g}") for g in range(NG)]
            sS5 = [small.tile([G, BL], F32, tag=f"sS5_{g}", name=f"sS5_{g}") for g in range(NG)]
            sS55 = [small.tile([G, BL], F32, tag=f"sS55_{g}", name=f"sS55_{g}") for g in range(NG)]
            sSxx = [small.tile([G, BL], F32, tag=f"sSxx_{g}", name=f"sSxx_{g}") for g in range(NG)]
            sS35 = [small.tile([G, BL], F32, tag=f"sS35_{g}", name=f"sS35_{g}") for g in range(NG)]
            sS3x = [small.tile([G, BL], F32, tag=f"sS3x_{g}", name=f"sS3x_{g}") for g in range(NG)]
            sS5x = [small.tile([G, BL], F32, tag=f"sS5x_{g}", name=f"sS5x_{g}") for g in range(NG)]

            def cross_sums(g):
                for b in range(BL):
                    xv = xps[g][b][:, 2:34, 2:34]
                    c3v = c3s[g][b][:].rearrange("p (h w) -> p h w", h=H)
                    c5v = c5s[g][b][:].rearrange("p (h w) -> p h w", h=H)
                    jd = junkd[:].rearrange("p (h w) -> p h w", h=H)
                    nc.scalar.activation(
                        junkf[:].rearrange("p (h w) -> p h w", h=H), xv,
                        ACT.Square, accum_out=sSxx[g][:, b:b + 1])
                    nc.vector.scalar_tensor_tensor(
                        junkd[:], c3s[g][b][:], 1.0, c5s[g][b][:],
                        ALU.bypass, ALU.mult, accum_out=sS35[g][:, b:b + 1])
                    nc.vector.scalar_tensor_tensor(
                        jd, xv, 1.0, c3v, ALU.bypass, ALU.mult,
                        accum_out=sS3x[g][:, b:b + 1])
                    nc.vector.scalar_tensor_tensor(
                        jd, xv, 1.0, c5v, ALU.bypass, ALU.mult,
                        accum_out=sS5x[g][:, b:b + 1])

            # ---------------- per-group: convs -> payload -> cc -> final --
            orr = out_ext.rearrange("b c h w -> c b (h w)")
            pays = [dram.tile([NPAY * G], F32, tag=f"pay{g}", name=f"pay{g}")
                    for g in range(NG)]
            prrs = [dram.tile([NPAY * G], F32, tag=f"prr{g}", name=f"prr{g}")
                    for g in range(NG)]

            def vtile(tag):
                return small.tile([G, 1], F32, tag=tag, name=tag)

            for g in (1, 0):
                cb = g * G
                if g == 0:
                    conv_pe(g, "c3", sS3[g], sS33[g])
                else:
                    conv_dve(g, "c3", sS3[g], sS33[g])
                conv_pe(g, "c5", sS5[g], sS55[g])
                cross_sums(g)

                # payload staging [G, NPAY]
                pstg = small.tile([G, NPAY], F32, tag=f"pstg{g}",
                                  name=f"pstg{g}")

                def put(col, src):
                    nc.vector.reduce_sum(pstg[:, CI[col]:CI[col] + 1], src,
                                         axis=AX.X)

                def putw(col, w, s):
                    wt = tv.tile([G, BL], F32, tag="wt", name="wt")
                    nc.vector.tensor_tensor(wt[:], w, s[:], ALU.mult)
                    put(col, wt[:])

                put("S3", sS3[g][:])
                put("S33", sS33[g][:])
                put("S5", sS5[g][:])
                put("S55", sS55[g][:])
                putw("uS3", kwu, sS3[g])
                putw("u2S3", u2[:], sS3[g])
                putw("uvS3", uv[:], sS3[g])
                putw("vS5", kwv, sS5[g])
                putw("v2S5", v2[:], sS5[g])
                putw("uvS5", uv[:], sS5[g])
                put("Sx", pooled[g][:])
                putw("uSx", kwu, pooled[g])
                putw("vSx", kwv, pooled[g])
                putw("u2S33", u2[:], sS33[g])
                putw("v2S55", v2[:], sS55[g])
                put("Sxx", sSxx[g][:])
                putw("uvS35", uv[:], sS35[g])
                putw("uS3x", kwu, sS3x[g])
                putw("vS5x", kwv, sS5x[g])
                put("U1", kwu)
                put("U2", u2[:])
                put("UV", uv[:])
                put("V1", kwv)
                put("V2", v2[:])

                pay_ap, prr_ap = pays[g][:], prrs[g][:]
                nc.sync.dma_start(
                    out=bass.AP(tensor=pay_ap.tensor, offset=pay_ap.offset,
                                ap=[[1, G], [G, NPAY]]),
                    in_=pstg[:])
                nc.gpsimd.collective_compute(
                    "AllReduce", ALU.add, replica_groups=RG,
                    ins=[pays[g][:].opt()], outs=[prrs[g][:].opt()])
                PG = small.tile([G, NPAY], F32, tag=f"PG{g}", name=f"PG{g}")
                nc.sync.dma_start(
                    out=PG[:],
                    in_=bass.AP(tensor=prr_ap.tensor, offset=prr_ap.offset,
                                ap=[[1, G], [G, NPAY]]))

                def pg(col):
                    return PG[:, CI[col]:CI[col] + 1]

                # ---- BN3/BN5 params, paired [G,2] (cols: conv3, conv5) ----
                mq = small.tile([G, 4], F32, tag="mq", name="mq")
                nc.vector.tensor_scalar(mq[:], PG[:, 0:4], INV_N, None, ALU.mult)
                mqv = mq[:].rearrange("p (a b) -> p a b", b=2)
                mpair = mqv[:, :, 0]
                qpair = mqv[:, :, 1]
                msq2 = small.tile([G, 2], F32, tag="msq2", name="msq2")
                nc.vector.tensor_tensor(msq2[:], mpair, mpair, ALU.mult)
                varp = small.tile([G, 2], F32, tag="varp", name="varp")
                nc.vector.tensor_tensor(varp[:], qpair, msq2[:], ALU.subtract)
                sdp = small.tile([G, 2], F32, tag="sdp", name="sdp")
                nc.scalar.activation(sdp[:], varp[:], ACT.Sqrt, bias=epst[:])
                rsp = small.tile([G, 2], F32, tag="rsp", name="rsp")
                nc.vector.reciprocal(rsp[:], sdp[:])
                gbv = pv[g][:, 0:4].rearrange("p (a b) -> p a b", b=2)
                apair = small.tile([G, 2], F32, tag="apair", name="apair")
                nc.vector.tensor_tensor(apair[:], rsp[:], gbv[:, :, 0], ALU.mult)
                tma = small.tile([G, 2], F32, tag="tma", name="tma")
                nc.vector.tensor_tensor(tma[:], mpair, apair[:], ALU.mult)
                dpair = small.tile([G, 2], F32, tag="dpair", name="dpair")
                nc.vector.tensor_tensor(dpair[:], gbv[:, :, 1], tma[:],
                                        ALU.subtract)
                a3, a5 = apair[:, 0:1], apair[:, 1:2]
                d3, d5 = dpair[:, 0:1], dpair[:, 1:2]
                w1 = pv[g][:, 4:5]
                b1c = pv[g][:, 5:6]

                def mul2(x, y, tag):
                    t = vtile(tag)
                    nc.vector.tensor_tensor(t[:], x, y, ALU.mult)
                    return t

                def fma(acc, in0, s):
                    # acc += in0 * s   (s: [G,1] AP or float)
                    nc.vector.scalar_tensor_tensor(acc[:], in0, s, acc[:],
                                                   ALU.mult, ALU.add)

                # row views of the payload (triples)
                X3, X1, X2 = PG[:, 4:7], PG[:, 7:10], PG[:, 10:13]
                Y1, Y2 = PG[:, 13:16], PG[:, 16:19]
                avec = small.tile([G, 3], F32, tag="avec", name="avec")
                nc.vector.tensor_copy(avec[:, 0:2], apair[:])
                nc.vector.tensor_copy(avec[:, 2:3], w1)

                # ---- Sout = dot(X3, avec) + HW*(d3 U1 + d5 V1 + B b1) ----
                sov = small.tile([G, 3], F32, tag="sov", name="sov")
                nc.vector.tensor_tensor(sov[:], X3, avec[:], ALU.mult)
                so_ = vtile("so_")
                nc.vector.reduce_sum(so_[:], sov[:], axis=AX.X)
                kt = vtile("kt")
                nc.vector.tensor_tensor(kt[:], d3, pg("U1"), ALU.mult)
                fma(kt, pg("V1"), d5)
                fma(kt, b1c, float(B))
                fma(so_, kt, float(HW))

                # ---- Sout2 = sum(Y1*avec^2 + 2*Y2*crvec + 2*t3vec*avec) + HW*K2
                sqv = small.tile([G, 3], F32, tag="sqv", name="sqv")
                nc.vector.tensor_tensor(sqv[:], avec[:], avec[:], ALU.mult)
                Z = small.tile([G, 3], F32, tag="Zv", name="Zv")
                nc.vector.tensor_tensor(Z[:], Y1, sqv[:], ALU.mult)
                crv = small.tile([G, 3], F32, tag="crv", name="crv")
                nc.vector.tensor_tensor(crv[:, 0:1], a3, a5, ALU.mult)
                nc.vector.tensor_tensor(crv[:, 1:2], a3, w1, ALU.mult)
                nc.vector.tensor_tensor(crv[:, 2:3], a5, w1, ALU.mult)
                cz = small.tile([G, 3], F32, tag="czv", name="czv")
                nc.vector.tensor_tensor(cz[:], Y2, crv[:], ALU.mult)
                nc.vector.scalar_tensor_tensor(Z[:], cz[:], 2.0, Z[:],
                                               ALU.mult, ALU.add)
                t3v = small.tile([G, 3], F32, tag="t3v", name="t3v")
                nc.vector.tensor_scalar(t3v[:], X1, d3, None, ALU.mult)
                nc.vector.scalar_tensor_tensor(t3v[:], X2, d5, t3v[:],
                                               ALU.mult, ALU.add)
                nc.vector.scalar_tensor_tensor(t3v[:], X3, b1c, t3v[:],
                                               ALU.mult, ALU.add)
                nc.vector.tensor_tensor(t3v[:], t3v[:], avec[:], ALU.mult)
                nc.vector.scalar_tensor_tensor(Z[:], t3v[:], 2.0, Z[:],
                                               ALU.mult, ALU.add)
                s2_ = vtile("s2_")
                nc.vector.reduce_sum(s2_[:], Z[:], axis=AX.X)
                # K^2 scalar part
                d3s = mul2(d3, d3, "d3s")
                d5s = mul2(d5, d5, "d5s")
                b1s = mul2(b1c, b1c, "b1s")
                d3d5 = mul2(d3, d5, "d3d5")
                d3b1 = mul2(d3, b1c, "d3b1")
                d5b1 = mul2(d5, b1c, "d5b1")
                t4 = vtile("t4_")
                nc.vector.tensor_tensor(t4[:], d3s[:], pg("U2"), ALU.mult)
                fma(t4, pg("V2"), d5s[:])
                fma(t4, b1s, float(B))
                t4b = vtile("t4b")
                nc.vector.tensor_tensor(t4b[:], d3d5[:], pg("UV"), ALU.mult)
                fma(t4b, pg("U1"), d3b1[:])
                fma(t4b, pg("V1"), d5b1[:])
                fma(t4, t4b, 2.0)
                fma(s2_, t4, float(HW))

                # ---- final affine params ----
                mO = vtile("mO_")
                nc.vector.tensor_scalar(mO[:], so_[:], INV_N, None, ALU.mult)
                qO = vtile("qO_")
                nc.vector.tensor_scalar(qO[:], s2_[:], INV_N, None, ALU.mult)
                mOsq = mul2(mO[:], mO[:], "mOsq")
                varO = vtile("varO")
                nc.vector.tensor_tensor(varO[:], qO[:], mOsq[:], ALU.subtract)
                sdO = vtile("sdO")
                nc.scalar.activation(sdO[:], varO[:], ACT.Sqrt, bias=epst[:])
                rsO = vtile("rsO")
                nc.vector.reciprocal(rsO[:], sdO[:])
                AO = mul2(rsO[:], pv[g][:, 6:7], "AO_")
                nAO = vtile("nAO")
                nc.vector.tensor_scalar(nAO[:], AO[:], -1.0, None, ALU.mult)
                DO = vtile("DO_")
                nc.vector.scalar_tensor_tensor(DO[:], mO[:], nAO[:],
                                               pv[g][:, 7:8], ALU.mult, ALU.add)

                AOa3 = mul2(AO[:], a3, "AOa3")
                AOa5 = mul2(AO[:], a5, "AOa5")
                AOw1 = mul2(AO[:], w1, "AOw1")
                AOd3 = mul2(AO[:], d3, "AOd3")
                AOd5 = mul2(AO[:], d5, "AOd5")
                cst0 = vtile("cst0")
                nc.vector.tensor_tensor(cst0[:], AO[:], b1c, ALU.mult)
                nc.vector.tensor_tensor(cst0[:], cst0[:], DO[:], ALU.add)

                alf3 = small.tile([G, BL], F32, tag=f"alf3_{g}", name=f"alf3_{g}")
                alf5 = small.tile([G, BL], F32, tag=f"alf5_{g}", name=f"alf5_{g}")
                dlt = small.tile([G, BL], F32, tag=f"dlt_{g}", name=f"dlt_{g}")
                nc.vector.tensor_scalar(alf3[:], kwu, AOa3[:], None, ALU.mult)
                nc.vector.tensor_scalar(alf5[:], kwv, AOa5[:], None, ALU.mult)
                nc.vector.tensor_scalar(dlt[:], kwu, AOd3[:], None, ALU.mult)
                nc.vector.scalar_tensor_tensor(dlt[:], kwv, AOd5[:], dlt[:],
                                               ALU.mult, ALU.add)
                nc.vector.tensor_scalar(dlt[:], dlt[:], 1.0, cst0[:],
                                        ALU.mult, ALU.add)

                # ---- final: out = alf3*c3 + alf5*c5 + AOw1*x + dlt ----
                # g1 (hidden under g0 convs): DVE path. g0 (the tail): PE
                # diag-matmuls — the conv psum pool is free by then.
                if g == 0:
                    dgw = small.tile([G, G], BF16, tag="dgw", name="dgw")
                    nc.vector.tensor_scalar(dgw[:], id128, AOw1[:], None,
                                            ALU.mult)
                for b in range(BL):
                    if g == 0:
                        dga = small.tile([G, G], BF16, tag="dga", name="dga")
                        dgb = small.tile([G, G], BF16, tag="dgb", name="dgb")
                        nc.vector.tensor_scalar(dga[:], id128,
                                                alf3[:, b:b + 1], None,
                                                ALU.mult)
                        nc.vector.tensor_scalar(dgb[:], id128,
                                                alf5[:, b:b + 1], None,
                                                ALU.mult)
                        ps = cpsum.tile([G, HW], F32, tag="cps", name="cps")
                        for half in range(2):
                            c0, c1 = half * 512, (half + 1) * 512
                            r0 = half * 16
                            nc.tensor.matmul(ps[:, c0:c1], dga[:],
                                             c3s[g][b][:, c0:c1],
                                             start=True, stop=False,
                                             skip_group_check=True)
                            nc.tensor.matmul(ps[:, c0:c1], dgb[:],
                                             c5s[g][b][:, c0:c1],
                                             start=False, stop=False,
                                             skip_group_check=True)
                            nc.tensor.matmul(ps[:, c0:c1], dgw[:],
                                             xps[g][b][:, 2 + r0:18 + r0, 2:34],
                                             start=False, stop=True,
                                             skip_group_check=True)
                        fstg = tb.tile([G, HW], F32, tag="fstg", name="fstg")
                        nc.scalar.activation(fstg[:], ps[:], ACT.Identity,
                                             bias=dlt[:, b:b + 1])
                        nc.sync.dma_start(out=orr[cb:cb + G, b], in_=fstg[:])
                        continue
                    t1 = tb.tile([G, HW], F32, tag="tbuf", name="tbuf")
                    nc.scalar.activation(t1[:], c3s[g][b][:], ACT.Identity,
                                         bias=dlt[:, b:b + 1],
                                         scale=alf3[:, b:b + 1])
                    nc.vector.scalar_tensor_tensor(t1[:], c5s[g][b][:],
                                                   alf5[:, b:b + 1], t1[:],
                                                   ALU.mult, ALU.add)
                    fstg = tb.tile([G, HW], F32, tag="fstg", name="fstg")
                    nc.vector.scalar_tensor_tensor(
                        fstg[:].rearrange("p (h w) -> p h w", h=H),
                        xps[g][b][:, 2:34, 2:34], AOw1[:],
                        t1[:].rearrange("p (h w) -> p h w", h=H),
                        ALU.mult, ALU.add)
                    nc.sync.dma_start(out=orr[cb:cb + G, b], in_=fstg[:])

    nc.compile()
    return nc


def kernel(**inputs):
    if "nc" not in _BUILT:
        _BUILT["nc"] = _build()
    nc = _BUILT["nc"]

    x = np.ascontiguousarray(inputs["x"], dtype=np.float32)
    k3 = inputs["conv3_w"].reshape(C, 9)
    k5 = inputs["conv5_w"].reshape(C, 25)
    pvec = np.stack([
        inputs["bn3_g"], inputs["bn3_b"], inputs["bn5_g"], inputs["bn5_b"],
        inputs["conv1_w"].reshape(C), inputs["conv1_b"],
        inputs["bn_g"], inputs["bn_b"]], axis=1)          # [C, 8]
    kp1s = np.asarray(inputs["kp1_w"]).T / float(HW)      # [C, 4]
    wps = []
    for g in range(NG):
        cb = g * G
        wps.append(np.concatenate(
            [k3[cb:cb + G], k5[cb:cb + G], kp1s[cb:cb + G], pvec[cb:cb + G]],
            axis=1).astype(np.float32))                   # [G, 46]
    wq = np.zeros((G, 144), np.float32)
    wq[:, 0:128] = np.eye(G, dtype=np.float32)            # id128
    wq[0:4, 136:138] = np.asarray(inputs["kp2_w"]).T      # kp2t
    wq[0:4, 138] = inputs["kpbn1_g"]                      # kbn col 0
    wq[0:4, 139] = inputs["kpbn1_b"]
    wq[0:2, 140] = inputs["kpbn2_g"]
    wq[0:2, 141] = inputs["kpbn2_b"]
    wq[0:2, 142:144] = np.eye(2, dtype=np.float32)        # id2

    in_maps = []
    for i in range(N_CORES):
        wqi = wq.copy()
        wqi[i * BL:(i + 1) * BL, 128:136] = np.eye(BL, dtype=np.float32)  # sel
        in_maps.append({
            "x": np.ascontiguousarray(x[i * BL:(i + 1) * BL]),
            "wp0": wps[0], "wp1": wps[1], "wq": wqi,
        })

    res = run_bass_kernel_spmd(nc, in_maps, list(range(N_CORES)))
    out = np.concatenate([res.results[i]["out"] for i in range(N_CORES)],
                         axis=0)
    return out

